# revision 4
# baseline (speedup 1.0000x reference)
"""Trainium2 Bass kernel for the AttentiveModule problem.

Reference computation (per batch element b, S=1024, D=512):
    att   = aspect @ inp.T / sqrt(len)                # [S,S]
    exp   = att * mask[:, None]                       # row mask (query dim)
    att_n = exp / (exp.sum(-1, keepdims=True) + 1e-4) # linear normalize
    w     = att_n @ inp                               # [S,D]
    ffn_inp = w + (inp + aspect) * mask[:, None]
    o1    = relu(ffn_inp @ w1.T + b1)
    o2    = relu(o1 @ w2.T + b2)
    final = 2*ffn_inp + o2
    out   = final / ||final||_2(axis=-1)

Sharding: data-parallel over batch, one batch element per NeuronCore (8 cores).

Key algebraic restructuring (all host-side prep is O(B*S*D), trivial vs the
O(S^2*D) device work):
  - The row mask and the linear normalization commute with the second matmul:
      w[s,:] = g[s] * (raw @ inp)[s,:]
    where raw = aspect @ inp.T (unmasked, unscaled) and
      g[s] = mask[s] / (mask[s]*rowsum_raw[s] + 1e-4*sqrt(len)).
    rowsum_raw[s] = aspect[s,:] @ inp.sum(0) is computed on the host in f64
    (it's a cheap matvec), so no [S,S] elementwise work or partition-dim
    reductions are needed on the device at all.
  - g is folded into the aspect operand of the first matmul (scaling column s
    of aspect.T scales column s of raw.T), so the first matmul directly
    produces att_n.T -- which is exactly the stationary operand layout the
    second matmul needs.
  - The factor 2 of `final = 2*ffn_inp + o2` is folded into g and the
    residual (host sends 2*(inp+aspect)*mask) and compensated with w1/2, so
    the device accumulates F2 = 2*ffn_inp directly and `final = F2 + o2`.
  - The residual add lands in PSUM via an fp32 identity matmul, the FFN2 bias
    via a K=1 ones x b2 matmul (a rank-1 broadcast add on the PE).

Matmul operands are bf16 (fp32 PSUM accumulation); the residual path
(resm2, F2, o2, final, output) stays fp32.
"""

import os
import sys

for _p in ("/opt/trn_rl_repo", "/opt/pypackages"):
    if os.path.isdir(_p) and _p not in sys.path:
        sys.path.append(_p)

import numpy as np
import ml_dtypes

BF16 = ml_dtypes.bfloat16

B, S, D = 8, 1024, 512
N_CORES = 8
P = 128                     # SBUF partitions
SB = S // P                 # 8 s-blocks of 128
DB = D // P                 # 4 d-blocks of 128
NF = 512                    # matmul moving free dim (one PSUM bank of fp32)
SH = S // NF                # 2 s-halves

_COMPILED = None


def _build():
    import concourse.bacc as bacc
    import concourse.tile as tile
    import concourse.mybir as mybir

    f32 = mybir.dt.float32
    bf16 = mybir.dt.bfloat16
    AF = mybir.ActivationFunctionType

    nc = bacc.Bacc("TRN2", target_bir_lowering=False, debug=False,
                   num_devices=N_CORES)

    # ---- I/O -------------------------------------------------------------
    inpT = nc.dram_tensor("inpT", [D, S], bf16, kind="ExternalInput").ap()
    aTg = nc.dram_tensor("aTg", [D, S], bf16, kind="ExternalInput").ap()
    inpN = nc.dram_tensor("inpN", [S, D], bf16, kind="ExternalInput").ap()
    resm2 = nc.dram_tensor("resm2", [S, D], f32, kind="ExternalInput").ap()
    w1th = nc.dram_tensor("w1th", [D, D], bf16, kind="ExternalInput").ap()
    w2t = nc.dram_tensor("w2t", [D, D], bf16, kind="ExternalInput").ap()
    b1cb = nc.dram_tensor("b1cb", [P, DB], f32, kind="ExternalInput").ap()
    b2r = nc.dram_tensor("b2r", [1, D], bf16, kind="ExternalInput").ap()
    onesr = nc.dram_tensor("onesr", [1, P], bf16, kind="ExternalInput").ap()
    ident = nc.dram_tensor("ident", [P, P], f32, kind="ExternalInput").ap()
    identb = nc.dram_tensor("identb", [P, P], bf16, kind="ExternalInput").ap()
    out = nc.dram_tensor("out", [S, D], f32, kind="ExternalOutput").ap()

    with tile.TileContext(nc) as tc:
        import contextlib
        ctx = contextlib.ExitStack()
        with ctx:
            consts = ctx.enter_context(tc.tile_pool(name="consts", bufs=1))
            big = ctx.enter_context(tc.tile_pool(name="big", bufs=1))
            psA = ctx.enter_context(tc.tile_pool(name="psA", bufs=2, space="PSUM"))
            psT = ctx.enter_context(tc.tile_pool(name="psT", bufs=2, space="PSUM"))
            work = ctx.enter_context(tc.tile_pool(name="work", bufs=2))

            # ---- constant / weight loads (small) -------------------------
            w1th_sb = []
            w2t_sb = []
            for db in range(DB):
                w1t_t = consts.tile([P, D], bf16, name=f"w1th_sb{db}")
                nc.sync.dma_start(w1t_t[:], w1th[db * P:(db + 1) * P, :])
                w1th_sb.append(w1t_t)
                w2t_t = consts.tile([P, D], bf16, name=f"w2t_sb{db}")
                nc.sync.dma_start(w2t_t[:], w2t[db * P:(db + 1) * P, :])
                w2t_sb.append(w2t_t)
            b1cb_sb = consts.tile([P, DB], f32, name="b1cb_sb")
            nc.sync.dma_start(b1cb_sb[:], b1cb[:])
            b2r_sb = consts.tile([1, D], bf16, name="b2r_sb")
            nc.sync.dma_start(b2r_sb[:], b2r[:])
            onesr_sb = consts.tile([1, P], bf16, name="onesr_sb")
            nc.sync.dma_start(onesr_sb[:], onesr[:])
            ident_sb = consts.tile([P, P], f32, name="ident_sb")
            nc.sync.dma_start(ident_sb[:], ident[:])
            identb_sb = consts.tile([P, P], bf16, name="identb_sb")
            nc.sync.dma_start(identb_sb[:], identb[:])

            # ---- big input loads ----------------------------------------
            inpT_sb = []
            aTg_sb = []
            for db in range(DB):
                it = big.tile([P, S], bf16, name=f"inpT_sb{db}")
                nc.sync.dma_start(it[:], inpT[db * P:(db + 1) * P, :])
                inpT_sb.append(it)
                at = big.tile([P, S], bf16, name=f"aTg_sb{db}")
                nc.sync.dma_start(at[:], aTg[db * P:(db + 1) * P, :])
                aTg_sb.append(at)
            inpN_sb = []
            resm2_sb = []
            for sb in range(SB):
                t = big.tile([P, D], bf16, name=f"inpN_sb{sb}")
                nc.sync.dma_start(t[:], inpN[sb * P:(sb + 1) * P, :])
                inpN_sb.append(t)
                r = big.tile([P, D], f32, name=f"resm2_sb{sb}")
                nc.sync.dma_start(r[:], resm2[sb * P:(sb + 1) * P, :])
                resm2_sb.append(r)

            # ---- phase A: attnT = (g * att_n).T  [t, s] ------------------
            # attnT[t,s] = sum_d inpT[d,t] * aTg[d,s]
            attnT_sb = []
            for tb in range(SB):
                at_t = big.tile([P, S], bf16, name=f"attnT_sb{tb}")
                attnT_sb.append(at_t)
            for tb in range(SB):
                for h in range(SH):
                    ps = psA.tile([P, NF], f32, name="psA_t", tag="psA")
                    for db in range(DB):
                        nc.tensor.matmul(
                            ps[:],
                            inpT_sb[db][:, tb * P:(tb + 1) * P],
                            aTg_sb[db][:, h * NF:(h + 1) * NF],
                            start=(db == 0),
                            stop=(db == DB - 1),
                        )
                    nc.scalar.activation(
                        attnT_sb[tb][:, h * NF:(h + 1) * NF], ps[:], AF.Copy)

            # ---- phase B: F2 = 2*ffn_inp = attnT.T @ inpN + resm2 --------
            F2_sb = []
            F2b_sb = []
            for sb in range(SB):
                f2 = big.tile([P, D], f32, name=f"F2_sb{sb}")
                F2_sb.append(f2)
                f2b = big.tile([P, D], bf16, name=f"F2b_sb{sb}")
                F2b_sb.append(f2b)
            for sb in range(SB):
                ps = psA.tile([P, NF], f32, name="psB_t", tag="psA")
                # residual enters PSUM through an identity matmul (fp32)
                nc.tensor.matmul(ps[:], ident_sb[:], resm2_sb[sb][:],
                                 start=True, stop=False)
                for tb in range(SB):
                    nc.tensor.matmul(
                        ps[:],
                        attnT_sb[tb][:, sb * P:(sb + 1) * P],
                        inpN_sb[tb][:],
                        start=False,
                        stop=(tb == SB - 1),
                    )
                nc.scalar.activation(F2_sb[sb][:], ps[:], AF.Copy)
                nc.vector.tensor_copy(F2b_sb[sb][:], F2_sb[sb][:])

            # ---- phase C: ffnT2 = F2.T (PE transposes, bf16) -------------
            ffnT2_sb = []
            for db in range(DB):
                t = big.tile([P, S], bf16, name=f"ffnT2_sb{db}")
                ffnT2_sb.append(t)
            for db in range(DB):
                for sb in range(SB):
                    pst = psT.tile([P, P], bf16, name="psT_t", tag="psT")
                    nc.tensor.transpose(
                        pst[:], F2b_sb[sb][:, db * P:(db + 1) * P], identb_sb[:])
                    if (sb % 2) == 0:
                        nc.scalar.activation(
                            ffnT2_sb[db][:, sb * P:(sb + 1) * P], pst[:], AF.Copy)
                    else:
                        nc.vector.tensor_copy(
                            ffnT2_sb[db][:, sb * P:(sb + 1) * P], pst[:])

            # ---- phase D: o1T = relu(w1th.T @ ffnT2 + b1) [e, s] ---------
            o1T_sb = []
            for eb in range(DB):
                t = big.tile([P, S], bf16, name=f"o1T_sb{eb}")
                o1T_sb.append(t)
            for eb in range(DB):
                for h in range(SH):
                    ps = psA.tile([P, NF], f32, name="psD_t", tag="psA")
                    for db in range(DB):
                        nc.tensor.matmul(
                            ps[:],
                            w1th_sb[db][:, eb * P:(eb + 1) * P],
                            ffnT2_sb[db][:, h * NF:(h + 1) * NF],
                            start=(db == 0),
                            stop=(db == DB - 1),
                        )
                    nc.scalar.activation(
                        o1T_sb[eb][:, h * NF:(h + 1) * NF], ps[:], AF.Relu,
                        bias=b1cb_sb[:, eb:eb + 1], scale=1.0)

            # ---- phase E+F: o2, final, normalize, store ------------------
            for sb in range(SB):
                ps = psA.tile([P, NF], f32, name="psE_t", tag="psA")
                # bias: rank-1 ones.T @ b2 broadcast (K=1 matmul)
                nc.tensor.matmul(ps[:], onesr_sb[:], b2r_sb[:],
                                 start=True, stop=False)
                for eb in range(DB):
                    nc.tensor.matmul(
                        ps[:],
                        o1T_sb[eb][:, sb * P:(sb + 1) * P],
                        w2t_sb[eb][:],
                        start=False,
                        stop=(eb == DB - 1),
                    )
                o2 = work.tile([P, D], f32, name="o2_t", tag="o2")
                nc.scalar.activation(o2[:], ps[:], AF.Relu)
                fin = work.tile([P, D], f32, name="fin_t", tag="fin")
                nc.vector.tensor_add(fin[:], o2[:], F2_sb[sb][:])
                # sum of squares via activation accumulate
                sq = work.tile([P, D], f32, name="sq_t", tag="sq")
                ss = work.tile([P, 1], f32, name="ss_t", tag="ss")
                nc.scalar.activation(sq[:], fin[:], AF.Square, accum_out=ss[:])
                rn = work.tile([P, 1], f32, name="rn_t", tag="rn")
                nc.scalar.activation(rn[:], ss[:], AF.Sqrt)
                rr = work.tile([P, 1], f32, name="rr_t", tag="rr")
                nc.vector.reciprocal(rr[:], rn[:])
                ot = work.tile([P, D], f32, name="ot_t", tag="ot")
                nc.vector.tensor_scalar_mul(ot[:], fin[:], rr[:])
                nc.sync.dma_start(out[sb * P:(sb + 1) * P, :], ot[:])

    nc.compile()
    return nc


def _get_compiled():
    global _COMPILED
    if _COMPILED is None:
        _COMPILED = _build()
    return _COMPILED


def _host_prep(inp, inp_len, aspect, w1, b1, w2, b2):
    inp = np.asarray(inp, dtype=np.float32)
    aspect = np.asarray(aspect, dtype=np.float32)
    inp_len = np.asarray(inp_len, dtype=np.float32)
    w1 = np.asarray(w1, dtype=np.float32)
    b1 = np.asarray(b1, dtype=np.float32)
    w2 = np.asarray(w2, dtype=np.float32)
    b2 = np.asarray(b2, dtype=np.float32)

    # ---- shared (replicated) host prep ----------------------------------
    w1th_np = np.ascontiguousarray((w1.T * 0.5).astype(BF16))   # [d, e] = w1.T/2
    w2t_np = np.ascontiguousarray(w2.T.astype(BF16))            # [e, f] = w2.T
    b1cb_np = np.ascontiguousarray(b1.reshape(DB, P).T.astype(np.float32))
    b2r_np = b2.reshape(1, D).astype(BF16)
    onesr_np = np.ones((1, P), dtype=BF16)
    ident_np = np.eye(P, dtype=np.float32)
    identb_np = np.eye(P).astype(BF16)

    in_maps = []
    for bidx in range(B):
        x = inp[bidx].astype(np.float64)          # [S, D]
        a = aspect[bidx].astype(np.float64)       # [S, D]
        ln = float(inp_len[bidx])
        scale = np.sqrt(ln)
        mask = (np.arange(S) < int(ln)).astype(np.float64)      # [S]
        rowsum = a @ x.sum(axis=0)                              # [S]
        g = mask / (mask * rowsum + 1e-4 * scale)               # [S]
        aTg2 = (a * (2.0 * g)[:, None]).T                       # [D, S]
        resm2 = 2.0 * (x + a) * mask[:, None]                   # [S, D]
        in_maps.append({
            "inpT": np.ascontiguousarray(x.T).astype(BF16),
            "aTg": np.ascontiguousarray(aTg2).astype(BF16),
            "inpN": x.astype(BF16),
            "resm2": resm2.astype(np.float32),
            "w1th": w1th_np,
            "w2t": w2t_np,
            "b1cb": b1cb_np,
            "b2r": b2r_np,
            "onesr": onesr_np,
            "ident": ident_np,
            "identb": identb_np,
        })
    return in_maps


def kernel(inp, inp_len, aspect, w1, b1, w2, b2):
    from concourse.bass_utils import run_bass_kernel_spmd

    nc = _get_compiled()
    in_maps = _host_prep(inp, inp_len, aspect, w1, b1, w2, b2)
    res = run_bass_kernel_spmd(nc, in_maps, core_ids=list(range(N_CORES)))
    return np.stack([res.results[i]["out"] for i in range(N_CORES)], axis=0)


# revision 5
# speedup vs baseline: 955.9791x; 955.9791x over previous
"""Trainium2 Bass kernel for the AttentiveModule problem.

Reference computation (per batch element b, S=1024, D=512):
    att   = aspect @ inp.T / sqrt(len)                # [S,S]
    exp   = att * mask[:, None]                       # row mask (query dim)
    att_n = exp / (exp.sum(-1, keepdims=True) + 1e-4) # linear normalize
    w     = att_n @ inp                               # [S,D]
    ffn_inp = w + (inp + aspect) * mask[:, None]
    o1    = relu(ffn_inp @ w1.T + b1)
    o2    = relu(o1 @ w2.T + b2)
    final = 2*ffn_inp + o2
    out   = final / ||final||_2(axis=-1)

Sharding: data-parallel over batch, one batch element per NeuronCore (8 cores).

Key algebraic restructuring (host prep is O(B*S*D), trivial vs the O(S^2*D)
device work):
  - The row mask and the linear normalization commute with the second matmul:
      w[s,:] = g[s] * (raw @ inp)[s,:],  raw = aspect @ inp.T
      g[s] = mask[s] / (mask[s]*rowsum_raw[s] + 1e-4*sqrt(len))
    rowsum_raw[s] = aspect[s,:] @ inp.sum(0) is a host-side f64 matvec, so the
    device needs no [S,S] elementwise work nor partition-dim reductions.
  - g (and the factor 2 of final = 2*ffn_inp + o2) is folded into the aspect
    operand of the first matmul, whose output attnT = (2*g*att_n).T lands
    directly in the stationary-operand layout the second matmul needs.
  - The residual 2*(inp+aspect)*mask enters PSUM via an fp32 identity matmul;
    the FFN2 bias via a K=1 ones x b2 matmul; w1/2 compensates the factor 2.
  - All inputs are packed on the host into three [128, X] images that match
    the SBUF destination layout exactly -> 3 big clean input DMAs.

Matmul operands are bf16 (fp32 PSUM accumulation); the residual path stays
fp32 end to end.
"""

import os
import sys

for _p in ("/opt/trn_rl_repo", "/opt/pypackages"):
    if os.path.isdir(_p) and _p not in sys.path:
        sys.path.append(_p)

import numpy as np
import ml_dtypes

BF16 = ml_dtypes.bfloat16

B, S, D = 8, 1024, 512
N_CORES = 8
P = 128                     # SBUF partitions
SB = S // P                 # 8 s-blocks of 128
DB = D // P                 # 4 d-blocks of 128
NF = 512                    # matmul moving free dim (one fp32 PSUM bank)
SH = S // NF                # 2 s-halves

# --- packed input column offsets (in elements) ---------------------------
# gA (bf16): inpT | aTg, both db-major [db*S + s]
A_INPT = 0
A_ATG = DB * S              # 4096
A_COLS = 2 * DB * S         # 8192
# gB (bf16): inpN (sb-major) | w1th | w2t | b2 row | ones row | identb
B_INPN = 0                  # 8 tiles [P, D] -> sb*D
B_W1 = SB * D               # 4096
B_W2 = B_W1 + DB * D        # 6144
B_B2 = B_W2 + DB * D        # 8192  (row 0 only)
B_ONES = B_B2 + D           # 8704  (row 0 only)
B_IDB = B_ONES + P          # 8832
B_COLS = B_IDB + P          # 8960
# gF (f32): resm2 (sb-major) | b1cb | ident
F_RES = 0
F_B1 = SB * D               # 4096
F_ID = F_B1 + DB            # 4100
F_COLS = F_ID + P           # 4228

_COMPILED = None


def _build():
    import concourse.bacc as bacc
    import concourse.tile as tile
    import concourse.mybir as mybir

    f32 = mybir.dt.float32
    bf16 = mybir.dt.bfloat16
    AF = mybir.ActivationFunctionType

    nc = bacc.Bacc("TRN2", target_bir_lowering=False, debug=False,
                   num_devices=N_CORES)

    packA = nc.dram_tensor("packA", [P, A_COLS], bf16, kind="ExternalInput").ap()
    packB = nc.dram_tensor("packB", [P, B_COLS], bf16, kind="ExternalInput").ap()
    packF = nc.dram_tensor("packF", [P, F_COLS], f32, kind="ExternalInput").ap()
    out = nc.dram_tensor("out", [S, D], f32, kind="ExternalOutput").ap()

    with tile.TileContext(nc) as tc:
        import contextlib
        ctx = contextlib.ExitStack()
        with ctx:
            big = ctx.enter_context(tc.tile_pool(name="big", bufs=1))
            psA = ctx.enter_context(tc.tile_pool(name="psA", bufs=3, space="PSUM"))
            psT = ctx.enter_context(tc.tile_pool(name="psT", bufs=2, space="PSUM"))
            work = ctx.enter_context(tc.tile_pool(name="work", bufs=2))

            gA = big.tile([P, A_COLS], bf16, name="gA")
            nc.sync.dma_start(gA[:], packA[:])
            gB = big.tile([P, B_COLS], bf16, name="gB")
            nc.sync.dma_start(gB[:], packB[:])
            gF = big.tile([P, F_COLS], f32, name="gF")
            nc.sync.dma_start(gF[:], packF[:])

            def inpT(db):           # [P, S] slice for d-block db
                return gA[:, A_INPT + db * S: A_INPT + (db + 1) * S]

            def aTg(db):
                return gA[:, A_ATG + db * S: A_ATG + (db + 1) * S]

            def inpN(sb):           # [P, D]
                return gB[:, B_INPN + sb * D: B_INPN + (sb + 1) * D]

            def w1th(db):           # [P, D]
                return gB[:, B_W1 + db * D: B_W1 + (db + 1) * D]

            def w2t(eb):
                return gB[:, B_W2 + eb * D: B_W2 + (eb + 1) * D]

            b2row = gB[0:1, B_B2: B_B2 + D]
            onesrow = gB[0:1, B_ONES: B_ONES + P]
            identb = gB[:, B_IDB: B_IDB + P]

            def resm2(sb):          # [P, D] f32
                return gF[:, F_RES + sb * D: F_RES + (sb + 1) * D]

            def b1col(eb):          # [P, 1] f32
                return gF[:, F_B1 + eb: F_B1 + eb + 1]

            ident = gF[:, F_ID: F_ID + P]

            # ---- phase A: attnT[t,s] = sum_d inpT[d,t] * aTg[d,s] --------
            attnT_sb = []
            for tb in range(SB):
                at_t = big.tile([P, S], bf16, name=f"attnT_sb{tb}")
                attnT_sb.append(at_t)
            for tb in range(SB):
                ps = psA.tile([P, S], f32, name="psA_t", tag="psA")
                for h in range(SH):
                    for db in range(DB):
                        nc.tensor.matmul(
                            ps[:, h * NF:(h + 1) * NF],
                            inpT(db)[:, tb * P:(tb + 1) * P],
                            aTg(db)[:, h * NF:(h + 1) * NF],
                            start=(db == 0),
                            stop=(db == DB - 1),
                        )
                if tb % 2 == 0:
                    nc.scalar.activation(attnT_sb[tb][:], ps[:], AF.Copy)
                else:
                    nc.vector.tensor_copy(attnT_sb[tb][:], ps[:])

            # ---- phase B: F2 = 2*ffn_inp = attnT.T @ inpN + resm2 --------
            F2_sb = []
            F2b_sb = []
            for sb in range(SB):
                f2 = big.tile([P, D], f32, name=f"F2_sb{sb}")
                F2_sb.append(f2)
                f2b = big.tile([P, D], bf16, name=f"F2b_sb{sb}")
                F2b_sb.append(f2b)
            for sb in range(SB):
                ps = psA.tile([P, NF], f32, name="psB_t", tag="psA")
                nc.tensor.matmul(ps[:], ident, resm2(sb), start=True, stop=False)
                for tb in range(SB):
                    nc.tensor.matmul(
                        ps[:],
                        attnT_sb[tb][:, sb * P:(sb + 1) * P],
                        inpN(tb),
                        start=False,
                        stop=(tb == SB - 1),
                    )
                nc.scalar.activation(F2_sb[sb][:], ps[:], AF.Copy)
                nc.vector.tensor_copy(F2b_sb[sb][:], F2_sb[sb][:])

            # ---- phase C: ffnT2 = F2.T (PE transposes, bf16) -------------
            ffnT2_sb = []
            for db in range(DB):
                t = big.tile([P, S], bf16, name=f"ffnT2_sb{db}")
                ffnT2_sb.append(t)
            for db in range(DB):
                for sb in range(SB):
                    pst = psT.tile([P, P], bf16, name="psT_t", tag="psT")
                    nc.tensor.transpose(
                        pst[:], F2b_sb[sb][:, db * P:(db + 1) * P], identb)
                    if (sb % 2) == 0:
                        nc.scalar.activation(
                            ffnT2_sb[db][:, sb * P:(sb + 1) * P], pst[:], AF.Copy)
                    else:
                        nc.vector.tensor_copy(
                            ffnT2_sb[db][:, sb * P:(sb + 1) * P], pst[:])

            # ---- phase D: o1T = relu(w1th.T @ ffnT2 + b1) [e, s] ---------
            o1T_sb = []
            for eb in range(DB):
                t = big.tile([P, S], bf16, name=f"o1T_sb{eb}")
                o1T_sb.append(t)
            for eb in range(DB):
                for h in range(SH):
                    ps = psA.tile([P, NF], f32, name="psD_t", tag="psA")
                    for db in range(DB):
                        nc.tensor.matmul(
                            ps[:],
                            w1th(db)[:, eb * P:(eb + 1) * P],
                            ffnT2_sb[db][:, h * NF:(h + 1) * NF],
                            start=(db == 0),
                            stop=(db == DB - 1),
                        )
                    nc.scalar.activation(
                        o1T_sb[eb][:, h * NF:(h + 1) * NF], ps[:], AF.Relu,
                        bias=b1col(eb), scale=1.0)

            # ---- phase E: o2, final, normalize, store --------------------
            for sb in range(SB):
                ps = psA.tile([P, NF], f32, name="psE_t", tag="psA")
                nc.tensor.matmul(ps[:], onesrow, b2row, start=True, stop=False)
                for eb in range(DB):
                    nc.tensor.matmul(
                        ps[:],
                        o1T_sb[eb][:, sb * P:(sb + 1) * P],
                        w2t(eb),
                        start=False,
                        stop=(eb == DB - 1),
                    )
                o2 = work.tile([P, D], f32, name="o2_t", tag="o2")
                nc.scalar.activation(o2[:], ps[:], AF.Relu)
                fin = work.tile([P, D], f32, name="fin_t", tag="fin")
                nc.vector.tensor_add(fin[:], o2[:], F2_sb[sb][:])
                sq = work.tile([P, D], f32, name="sq_t", tag="sq")
                ss = work.tile([P, 1], f32, name="ss_t", tag="ss")
                nc.scalar.activation(sq[:], fin[:], AF.Square, accum_out=ss[:])
                rn = work.tile([P, 1], f32, name="rn_t", tag="rn")
                nc.scalar.activation(rn[:], ss[:], AF.Sqrt)
                rr = work.tile([P, 1], f32, name="rr_t", tag="rr")
                nc.vector.reciprocal(rr[:], rn[:])
                ot = work.tile([P, D], f32, name="ot_t", tag="ot")
                nc.vector.tensor_scalar_mul(ot[:], fin[:], rr[:])
                nc.sync.dma_start(out[sb * P:(sb + 1) * P, :], ot[:])

    nc.compile()
    return nc


def _get_compiled():
    global _COMPILED
    if _COMPILED is None:
        _COMPILED = _build()
    return _COMPILED


def _host_prep(inp, inp_len, aspect, w1, b1, w2, b2):
    inp = np.asarray(inp, dtype=np.float32)
    aspect = np.asarray(aspect, dtype=np.float32)
    inp_len = np.asarray(inp_len, dtype=np.float32)
    w1 = np.asarray(w1, dtype=np.float32)
    b1 = np.asarray(b1, dtype=np.float32)
    w2 = np.asarray(w2, dtype=np.float32)
    b2 = np.asarray(b2, dtype=np.float32)

    # shared pieces of packB / packF
    packB_shared = np.zeros((P, B_COLS), dtype=BF16)
    w1th = (w1.T * 0.5).astype(BF16)                 # [d, e]
    w2t = w2.T.astype(BF16)                          # [e, f]
    for db in range(DB):
        packB_shared[:, B_W1 + db * D: B_W1 + (db + 1) * D] = \
            w1th[db * P:(db + 1) * P, :]
        packB_shared[:, B_W2 + db * D: B_W2 + (db + 1) * D] = \
            w2t[db * P:(db + 1) * P, :]
    packB_shared[0, B_B2: B_B2 + D] = b2.astype(BF16)
    packB_shared[0, B_ONES: B_ONES + P] = np.ones(P, dtype=BF16)
    packB_shared[:, B_IDB: B_IDB + P] = np.eye(P).astype(BF16)

    b1cb = b1.reshape(DB, P).T.astype(np.float32)    # [P, DB]
    ident = np.eye(P, dtype=np.float32)

    in_maps = []
    for bidx in range(B):
        x = inp[bidx].astype(np.float64)             # [S, D]
        a = aspect[bidx].astype(np.float64)
        ln = float(inp_len[bidx])
        scale = np.sqrt(ln)
        mask = (np.arange(S) < int(ln)).astype(np.float64)
        rowsum = a @ x.sum(axis=0)
        g = mask / (mask * rowsum + 1e-4 * scale)
        aTg2 = (a * (2.0 * g)[:, None]).T            # [D, S]
        xT = x.T                                     # [D, S]
        resm2 = 2.0 * (x + a) * mask[:, None]        # [S, D]

        pA = np.empty((P, A_COLS), dtype=BF16)
        for db in range(DB):
            pA[:, A_INPT + db * S: A_INPT + (db + 1) * S] = \
                xT[db * P:(db + 1) * P, :].astype(BF16)
            pA[:, A_ATG + db * S: A_ATG + (db + 1) * S] = \
                aTg2[db * P:(db + 1) * P, :].astype(BF16)

        pB = packB_shared.copy()
        xb = x.astype(BF16)
        for sb in range(SB):
            pB[:, B_INPN + sb * D: B_INPN + (sb + 1) * D] = \
                xb[sb * P:(sb + 1) * P, :]

        pF = np.zeros((P, F_COLS), dtype=np.float32)
        r32 = resm2.astype(np.float32)
        for sb in range(SB):
            pF[:, F_RES + sb * D: F_RES + (sb + 1) * D] = \
                r32[sb * P:(sb + 1) * P, :]
        pF[:, F_B1: F_B1 + DB] = b1cb
        pF[:, F_ID: F_ID + P] = ident

        in_maps.append({"packA": pA, "packB": pB, "packF": pF})
    return in_maps


def kernel(inp, inp_len, aspect, w1, b1, w2, b2):
    from concourse.bass_utils import run_bass_kernel_spmd

    nc = _get_compiled()
    in_maps = _host_prep(inp, inp_len, aspect, w1, b1, w2, b2)
    res = run_bass_kernel_spmd(nc, in_maps, core_ids=list(range(N_CORES)))
    return np.stack([res.results[i]["out"] for i in range(N_CORES)], axis=0)


# revision 6
# speedup vs baseline: 1125.7189x; 1.1776x over previous
"""Trainium2 Bass kernel for the AttentiveModule problem.

Reference computation (per batch element b, S=1024, D=512):
    att   = aspect @ inp.T / sqrt(len)                # [S,S]
    exp   = att * mask[:, None]                       # row mask (query dim)
    att_n = exp / (exp.sum(-1, keepdims=True) + 1e-4) # linear normalize
    w     = att_n @ inp                               # [S,D]
    ffn_inp = w + (inp + aspect) * mask[:, None]
    o1    = relu(ffn_inp @ w1.T + b1)
    o2    = relu(o1 @ w2.T + b2)
    final = 2*ffn_inp + o2
    out   = final / ||final||_2(axis=-1)

Sharding: data-parallel over batch, one batch element per NeuronCore (8 cores).

Key algebraic restructuring (host prep is O(B*S*D), trivial vs the O(S^2*D)
device work):
  - The row mask and the linear normalization commute with the second matmul:
      w[s,:] = g[s] * (raw @ inp)[s,:],  raw = aspect @ inp.T
      g[s] = mask[s] / (mask[s]*rowsum_raw[s] + 1e-4*sqrt(len))
    rowsum_raw[s] = aspect[s,:] @ inp.sum(0) is a host-side f64 matvec, so the
    device needs no [S,S] elementwise work nor partition-dim reductions.
  - g (and the factor 2 of final = 2*ffn_inp + o2) is folded into the aspect
    operand of the first matmul, whose output attnT = (2*g*att_n).T lands
    directly in the stationary-operand layout the second matmul needs.
  - The FFN2 bias enters PSUM via a K=1 ones x b2 matmul; w1/2 compensates
    the folded factor 2.
  - Inputs are packed on the host into [128, X] images matching the SBUF
    destination layout exactly -> a handful of big clean input DMAs, ordered
    so phase A's operands land first.

Matmul operands are bf16 (fp32 PSUM accumulation); the residual path stays
fp32 end to end.
"""

import os
import sys

for _p in ("/opt/trn_rl_repo", "/opt/pypackages"):
    if os.path.isdir(_p) and _p not in sys.path:
        sys.path.append(_p)

import numpy as np
import ml_dtypes

BF16 = ml_dtypes.bfloat16

B, S, D = 8, 1024, 512
N_CORES = 8
P = 128                     # SBUF partitions
SB = S // P                 # 8 s-blocks of 128
DB = D // P                 # 4 d-blocks of 128
NF = 512                    # matmul moving free dim (one fp32 PSUM bank)
SH = S // NF                # 2 s-halves

# --- packed input layouts (element column offsets) -----------------------
# gA (bf16) = inpT split in t-halves, aTg split in s-halves, interleaved so
# the first chunk pair unblocks phase A early:
#   chunk0: inpT t-half 0   (db-major, [P, 4*NF])
#   chunk1: aTg  s-half 0
#   chunk2: inpT t-half 1
#   chunk3: aTg  s-half 1
A_CH = DB * NF              # 2048 elements per chunk
A_COLS = 4 * A_CH
# gB1 (bf16): inpN (sb-major)
B1_COLS = SB * D            # 4096
# gB2 (bf16): w1th | w2t | b2 row | ones row | identb
B2_W1 = 0
B2_W2 = DB * D              # 2048
B2_B2 = B2_W2 + DB * D      # 4096  (row 0 only)
B2_ONES = B2_B2 + D         # 4608  (row 0 only)
B2_IDB = B2_ONES + P        # 4736
B2_COLS = B2_IDB + P        # 4864
# gF (f32): resm2 (sb-major) | b1cb
F_RES = 0
F_B1 = SB * D               # 4096
F_COLS = F_B1 + DB          # 4100

_COMPILED = None


def _build():
    import concourse.bacc as bacc
    import concourse.tile as tile
    import concourse.mybir as mybir

    f32 = mybir.dt.float32
    bf16 = mybir.dt.bfloat16
    AF = mybir.ActivationFunctionType

    nc = bacc.Bacc("TRN2", target_bir_lowering=False, debug=False,
                   num_devices=N_CORES)

    packA = nc.dram_tensor("packA", [P, A_COLS], bf16, kind="ExternalInput").ap()
    packB1 = nc.dram_tensor("packB1", [P, B1_COLS], bf16, kind="ExternalInput").ap()
    packB2 = nc.dram_tensor("packB2", [P, B2_COLS], bf16, kind="ExternalInput").ap()
    packF = nc.dram_tensor("packF", [P, F_COLS], f32, kind="ExternalInput").ap()
    out = nc.dram_tensor("out", [S, D], f32, kind="ExternalOutput").ap()

    with tile.TileContext(nc) as tc:
        import contextlib
        ctx = contextlib.ExitStack()
        with ctx:
            big = ctx.enter_context(tc.tile_pool(name="big", bufs=1))
            psA = ctx.enter_context(tc.tile_pool(name="psA", bufs=4, space="PSUM"))
            psT = ctx.enter_context(tc.tile_pool(name="psT", bufs=4, space="PSUM"))
            work = ctx.enter_context(tc.tile_pool(name="work", bufs=2))

            # phase-A-critical chunks first, in consumption order
            gA = big.tile([P, A_COLS], bf16, name="gA")
            for ch in range(4):
                nc.sync.dma_start(gA[:, ch * A_CH:(ch + 1) * A_CH],
                                  packA[:, ch * A_CH:(ch + 1) * A_CH])
            gB1 = big.tile([P, B1_COLS], bf16, name="gB1")
            nc.sync.dma_start(gB1[:], packB1[:])
            gF = big.tile([P, F_COLS], f32, name="gF")
            nc.sync.dma_start(gF[:], packF[:])
            gB2 = big.tile([P, B2_COLS], bf16, name="gB2")
            nc.sync.dma_start(gB2[:], packB2[:])

            def inpT(th, db):       # [P, NF]: inpT[d-block db, t-half th]
                off = (2 * th) * A_CH + db * NF
                return gA[:, off: off + NF]

            def aTg(h, db):         # [P, NF]: aTg[d-block db, s-half h]
                off = (2 * h + 1) * A_CH + db * NF
                return gA[:, off: off + NF]

            def inpN(sb):           # [P, D]
                return gB1[:, sb * D: (sb + 1) * D]

            def w1th(db):           # [P, D]
                return gB2[:, B2_W1 + db * D: B2_W1 + (db + 1) * D]

            def w2t(eb):
                return gB2[:, B2_W2 + eb * D: B2_W2 + (eb + 1) * D]

            b2row = gB2[0:1, B2_B2: B2_B2 + D]
            onesrow = gB2[0:1, B2_ONES: B2_ONES + P]
            identb = gB2[:, B2_IDB: B2_IDB + P]

            def resm2(sb):          # [P, D] f32
                return gF[:, F_RES + sb * D: F_RES + (sb + 1) * D]

            def b1col(eb):          # [P, 1] f32
                return gF[:, F_B1 + eb: F_B1 + eb + 1]

            # ---- phase A: attnT[t,s] = sum_d inpT[d,t] * aTg[d,s] --------
            attnT_sb = []
            for tb in range(SB):
                at_t = big.tile([P, S], bf16, name=f"attnT_sb{tb}")
                attnT_sb.append(at_t)
            gi = 0
            for th in range(2):
                for h in range(SH):
                    for ti in range(4):
                        tb = th * 4 + ti
                        ps = psA.tile([P, NF], f32, name="psA_t", tag="psA")
                        for db in range(DB):
                            nc.tensor.matmul(
                                ps[:],
                                inpT(th, db)[:, ti * P:(ti + 1) * P],
                                aTg(h, db),
                                start=(db == 0),
                                stop=(db == DB - 1),
                            )
                        dst = attnT_sb[tb][:, h * NF:(h + 1) * NF]
                        if gi % 2 == 0:
                            nc.scalar.activation(dst, ps[:], AF.Copy)
                        else:
                            nc.vector.tensor_copy(dst, ps[:])
                        gi += 1

            # ---- phase B + C interleaved ---------------------------------
            # B: F2 = 2*ffn_inp = attnT.T @ inpN + resm2  (add on DVE)
            # C: ffnT2 = F2.T via PE transposes, per s-tile as soon as F2b
            #    is ready
            F2_sb = []
            F2b_sb = []
            for sb in range(SB):
                f2 = big.tile([P, D], f32, name=f"F2_sb{sb}")
                F2_sb.append(f2)
                f2b = big.tile([P, D], bf16, name=f"F2b_sb{sb}")
                F2b_sb.append(f2b)
            ffnT2_sb = []
            for db in range(DB):
                t = big.tile([P, S], bf16, name=f"ffnT2_sb{db}")
                ffnT2_sb.append(t)
            for sb in range(SB):
                ps = psA.tile([P, NF], f32, name="psB_t", tag="psA")
                for tb in range(SB):
                    nc.tensor.matmul(
                        ps[:],
                        attnT_sb[tb][:, sb * P:(sb + 1) * P],
                        inpN(tb),
                        start=(tb == 0),
                        stop=(tb == SB - 1),
                    )
                nc.vector.tensor_add(F2_sb[sb][:], ps[:], resm2(sb))
                nc.vector.tensor_copy(F2b_sb[sb][:], F2_sb[sb][:])
                for db in range(DB):
                    pst = psT.tile([P, P], bf16, name="psT_t", tag="psT")
                    nc.tensor.transpose(
                        pst[:], F2b_sb[sb][:, db * P:(db + 1) * P], identb)
                    dst = ffnT2_sb[db][:, sb * P:(sb + 1) * P]
                    if (sb + db) % 2 == 0:
                        nc.scalar.activation(dst, pst[:], AF.Copy)
                    else:
                        nc.vector.tensor_copy(dst, pst[:])

            # ---- phase D: o1T = relu(w1th.T @ ffnT2 + b1) [e, s] ---------
            o1T_sb = []
            for eb in range(DB):
                t = big.tile([P, S], bf16, name=f"o1T_sb{eb}")
                o1T_sb.append(t)
            for eb in range(DB):
                for h in range(SH):
                    ps = psA.tile([P, NF], f32, name="psD_t", tag="psA")
                    for db in range(DB):
                        nc.tensor.matmul(
                            ps[:],
                            w1th(db)[:, eb * P:(eb + 1) * P],
                            ffnT2_sb[db][:, h * NF:(h + 1) * NF],
                            start=(db == 0),
                            stop=(db == DB - 1),
                        )
                    nc.scalar.activation(
                        o1T_sb[eb][:, h * NF:(h + 1) * NF], ps[:], AF.Relu,
                        bias=b1col(eb), scale=1.0)

            # ---- phase E: o2, final, normalize, store --------------------
            for sb in range(SB):
                ps = psA.tile([P, NF], f32, name="psE_t", tag="psA")
                nc.tensor.matmul(ps[:], onesrow, b2row, start=True, stop=False)
                for eb in range(DB):
                    nc.tensor.matmul(
                        ps[:],
                        o1T_sb[eb][:, sb * P:(sb + 1) * P],
                        w2t(eb),
                        start=False,
                        stop=(eb == DB - 1),
                    )
                o2 = work.tile([P, D], f32, name="o2_t", tag="o2")
                nc.scalar.activation(o2[:], ps[:], AF.Relu)
                fin = work.tile([P, D], f32, name="fin_t", tag="fin")
                nc.vector.tensor_add(fin[:], o2[:], F2_sb[sb][:])
                sq = work.tile([P, D], f32, name="sq_t", tag="sq")
                ss = work.tile([P, 1], f32, name="ss_t", tag="ss")
                nc.scalar.activation(sq[:], fin[:], AF.Square, accum_out=ss[:])
                rn = work.tile([P, 1], f32, name="rn_t", tag="rn")
                nc.scalar.activation(rn[:], ss[:], AF.Sqrt)
                rr = work.tile([P, 1], f32, name="rr_t", tag="rr")
                nc.vector.reciprocal(rr[:], rn[:])
                ot = work.tile([P, D], f32, name="ot_t", tag="ot")
                nc.vector.tensor_scalar_mul(ot[:], fin[:], rr[:])
                nc.sync.dma_start(out[sb * P:(sb + 1) * P, :], ot[:])

    nc.compile()
    return nc


def _get_compiled():
    global _COMPILED
    if _COMPILED is None:
        _COMPILED = _build()
    return _COMPILED


def _host_prep(inp, inp_len, aspect, w1, b1, w2, b2):
    inp = np.asarray(inp, dtype=np.float32)
    aspect = np.asarray(aspect, dtype=np.float32)
    inp_len = np.asarray(inp_len, dtype=np.float32)
    w1 = np.asarray(w1, dtype=np.float32)
    b1 = np.asarray(b1, dtype=np.float32)
    w2 = np.asarray(w2, dtype=np.float32)
    b2 = np.asarray(b2, dtype=np.float32)

    packB2 = np.zeros((P, B2_COLS), dtype=BF16)
    w1th = (w1.T * 0.5).astype(BF16)                 # [d, e]
    w2t = w2.T.astype(BF16)                          # [e, f]
    for db in range(DB):
        packB2[:, B2_W1 + db * D: B2_W1 + (db + 1) * D] = \
            w1th[db * P:(db + 1) * P, :]
        packB2[:, B2_W2 + db * D: B2_W2 + (db + 1) * D] = \
            w2t[db * P:(db + 1) * P, :]
    packB2[0, B2_B2: B2_B2 + D] = b2.astype(BF16)
    packB2[0, B2_ONES: B2_ONES + P] = np.ones(P, dtype=BF16)
    packB2[:, B2_IDB: B2_IDB + P] = np.eye(P).astype(BF16)

    b1cb = b1.reshape(DB, P).T.astype(np.float32)    # [P, DB]

    in_maps = []
    for bidx in range(B):
        x = inp[bidx].astype(np.float64)             # [S, D]
        a = aspect[bidx].astype(np.float64)
        ln = float(inp_len[bidx])
        scale = np.sqrt(ln)
        mask = (np.arange(S) < int(ln)).astype(np.float64)
        rowsum = a @ x.sum(axis=0)
        g = mask / (mask * rowsum + 1e-4 * scale)
        aTg2 = (a * (2.0 * g)[:, None]).T            # [D, S]
        xT = x.T                                     # [D, S]
        resm2 = 2.0 * (x + a) * mask[:, None]        # [S, D]

        pA = np.empty((P, A_COLS), dtype=BF16)
        for half in range(2):
            sl = slice(half * NF, (half + 1) * NF)
            for db in range(DB):
                pA[:, (2 * half) * A_CH + db * NF:
                      (2 * half) * A_CH + (db + 1) * NF] = \
                    xT[db * P:(db + 1) * P, sl].astype(BF16)
                pA[:, (2 * half + 1) * A_CH + db * NF:
                      (2 * half + 1) * A_CH + (db + 1) * NF] = \
                    aTg2[db * P:(db + 1) * P, sl].astype(BF16)

        pB1 = np.empty((P, B1_COLS), dtype=BF16)
        xb = x.astype(BF16)
        for sb in range(SB):
            pB1[:, sb * D:(sb + 1) * D] = xb[sb * P:(sb + 1) * P, :]

        pF = np.zeros((P, F_COLS), dtype=np.float32)
        r32 = resm2.astype(np.float32)
        for sb in range(SB):
            pF[:, F_RES + sb * D: F_RES + (sb + 1) * D] = \
                r32[sb * P:(sb + 1) * P, :]
        pF[:, F_B1: F_B1 + DB] = b1cb

        in_maps.append({"packA": pA, "packB1": pB1, "packB2": packB2,
                        "packF": pF})
    return in_maps


def kernel(inp, inp_len, aspect, w1, b1, w2, b2):
    from concourse.bass_utils import run_bass_kernel_spmd

    nc = _get_compiled()
    in_maps = _host_prep(inp, inp_len, aspect, w1, b1, w2, b2)
    res = run_bass_kernel_spmd(nc, in_maps, core_ids=list(range(N_CORES)))
    return np.stack([res.results[i]["out"] for i in range(N_CORES)], axis=0)
